# revision 19
# baseline (speedup 1.0000x reference)
"""Trainium2 Bass kernel for nn_ModelLSTM (36-step scalar-feedback LSTM).

Model (per reference):
    emb = relu(x @ W1.T + b1)                       # [B, 511], constant across steps
    x0 = -0.8; h0 = c0 = 0
    step t: inp = [emb, xin]                        # [B, 512]
            gates = inp @ Wih.T + bih + h @ Whh.T + bhh
            i,f,g,o = split(gates); c' = sig(f)*c + sig(i)*tanh(g); h' = sig(o)*tanh(c')
            y = h' @ W3.T + b3 ; xin' = y
    output ys = [36, B, 1]

Key restructurings:
  * xin_t = y_{t-1} = h_t @ W3.T + b3 folds into the recurrent weights:
    Whh_eff = Whh + Wih[:,511:] @ W3, bias folded similarly.
  * The emb contribution A = Wih[:,:511] @ emb.T is precomputed once and
    re-injected each step (identity-matmul into PSUM on PE, or DVE add).
  * fp8 (float8e4) DoubleRow matmuls for the i,f,o gate blocks (sigmoid's
    slope <= 1/4 attenuates the quantization error; the tanh g-block stays
    bf16). DoubleRow packs two 128-row contraction tiles per pass -> ~1.77x
    PE throughput on those tiles. Weights are pre-scaled by 32 to keep fp8
    values in the normal range; the gate activation applies scale=1/32.
    Host-side fp8 sweep predicts rel_err ~0.007 (vs 0.005 all-bf16).
  * c is stored bf16 so all cell-update DVE ops run in 2x packed mode.

Sharding: pure data-parallel over batch (16384 -> 8 cores x 2048). Weights
replicated. No collectives.

Layout: everything transposed - batch is the free dim, gate/hidden index is
the partition dim.
"""

import sys

for _p in ("/opt/trn_rl_repo",):
    if _p not in sys.path:
        sys.path.insert(0, _p)

import numpy as np
import ml_dtypes

BF16 = ml_dtypes.bfloat16
F8E4 = ml_dtypes.float8_e4m3  # TRN float8e4 (max 240)

N_CORES = 8
B = 16384
BL = B // N_CORES  # 2048 batch per core
HID = 512
EMB = 511
STEPS = 36
NG = 4 * HID  # 2048 gate rows
M_TILES = 16  # gate row tiles of 128
K_TILES = 4  # contraction tiles of 128 over HID
NC_CH = BL // 512  # 4 free-dim chunks of 512
SC = 32.0  # fp8 weight pre-scale (descaled in the gate activation)

FP8_GATES = (0, 1, 3)  # i, f, o via fp8 DoubleRow; g (tanh) stays bf16


def _build_program(
    b3_val: float,
    n_steps: int = STEPS,
    fp8_gates: tuple = FP8_GATES,
    ident_hids: tuple = (1, 3),
    gps_fc_hids: tuple = (0, 1),
    gps_ig_hids: tuple = (0,),
    split_last_cell: bool = True,
):
    """Build the Bass program.

    fp8_gates: gate types (0=i,1=f,2=g,3=o) whose matmul runs fp8 DoubleRow.
    ident_hids: hid groups whose A tiles are re-injected via identity matmul
      on PE; the rest add A on the Vector engine (PE vs DVE load balance).
    gps_fc_hids/gps_ig_hids: hids whose f*c / i*g products run on GpSimd.
    """
    import concourse.bass as bass
    import concourse.bacc as bacc
    import concourse.tile as tile
    from concourse import mybir

    fp32 = mybir.dt.float32
    bf16 = mybir.dt.bfloat16
    fp8 = mybir.dt.float8e4
    AF = mybir.ActivationFunctionType
    OP = mybir.AluOpType
    DR = mybir.MatmulPerfMode.DoubleRow
    ISC = 1.0 / SC

    F_MS = [m for m in range(M_TILES) if m // 4 in fp8_gates]  # fp8 m-tiles
    G_MS = [m for m in range(M_TILES) if m // 4 not in fp8_gates]  # bf16 m-tiles
    NF, NGB = len(F_MS), len(G_MS)
    fidx = {m: i for i, m in enumerate(F_MS)}
    gidx = {m: i for i, m in enumerate(G_MS)}
    Y_OFF = NGB * 128  # y column offset inside wbig

    nc = bacc.Bacc(
        "TRN2",
        target_bir_lowering=False,
        debug=False,
        num_devices=N_CORES,
    )

    # ---- DRAM I/O (per-core shapes) ----
    xT_d = nc.dram_tensor("xT", [24, BL], bf16, kind="ExternalInput")
    w1t_d = nc.dram_tensor("w1t", [24, 512], bf16, kind="ExternalInput")
    # bf16 lhsT: g-gate columns (x32) then the y column (x1), zero padded
    wbig_d = nc.dram_tensor("wbig", [HID, Y_OFF + 128], bf16, kind="ExternalInput")
    # fp8 DoubleRow weight pairs: pair p covers k-tiles (2p, 2p+1)
    w8_d = [
        nc.dram_tensor(f"w8p{p}", [128, 2, NF * 128], fp8, kind="ExternalInput")
        for p in range(2)
    ]
    we_d = nc.dram_tensor("we", [HID, NG], bf16, kind="ExternalInput")
    ident_d = nc.dram_tensor("ident", [128, 128], bf16, kind="ExternalInput")
    bias_st_d = nc.dram_tensor("bias_st", [128, M_TILES], fp32, kind="ExternalInput")
    bias_d0_d = nc.dram_tensor("bias_d0", [128, M_TILES], fp32, kind="ExternalInput")
    out_d = nc.dram_tensor("out", [STEPS, BL], fp32, kind="ExternalOutput")
    # scratch DRAM for the precomputed emb contribution A (streamed every step)
    a_d = nc.dram_tensor("a_scratch", [M_TILES, 128, BL], bf16, kind="Internal")

    with tile.TileContext(nc) as tc:
        with (
            tc.tile_pool(name="const", bufs=1) as constp,
            tc.tile_pool(name="state", bufs=1) as statep,
            tc.tile_pool(name="hpool", bufs=2) as hpool,
            tc.tile_pool(name="work", bufs=2) as workp,
            tc.tile_pool(name="astream", bufs=4) as apool,
            tc.tile_pool(name="psum", bufs=2, space=bass.MemorySpace.PSUM) as psump,
        ):
            # ---- load constants (emb inputs first so PE starts ASAP) ----
            xT_sb = workp.tile([24, BL], bf16, tag="fc", name="xT_sb")
            nc.sync.dma_start(xT_sb[:], xT_d[:])
            w1t_sb = workp.tile([24, 512], bf16, tag="ig", name="w1t_sb")
            nc.sync.dma_start(w1t_sb[:], w1t_d[:])
            wbig_sb = []
            for k in range(K_TILES):
                wt = constp.tile([128, Y_OFF + 128], bf16, name=f"wbig{k}")
                nc.sync.dma_start(wt[:], wbig_d[k * 128 : (k + 1) * 128, :])
                wbig_sb.append(wt)
            w8_sb = []
            for p in range(2):
                wt = constp.tile([128, 2, NF * 128], fp8, name=f"w8p{p}")
                nc.sync.dma_start(wt[:], w8_d[p][:])
                w8_sb.append(wt)
            ident_sb = constp.tile([128, 128], bf16, name="ident")
            nc.sync.dma_start(ident_sb[:], ident_d[:])
            bias_st = constp.tile([128, M_TILES], fp32, name="bias_st")
            nc.sync.dma_start(bias_st[:], bias_st_d[:])
            bias_d0 = constp.tile([128, M_TILES], fp32, name="bias_d0")
            nc.sync.dma_start(bias_d0[:], bias_d0_d[:])

            # ---- setup: emb = relu(x @ W1.T + b1) (transposed, bf16) ----
            we_sb = []
            for k in range(K_TILES):
                wet = hpool.tile([128, NG], bf16, tag=f"h{k}", name=f"we{k}")
                nc.sync.dma_start(wet[:], we_d[k * 128 : (k + 1) * 128, :])
                we_sb.append(wet)

            embT = []
            for mj in range(4):
                eps = psump.tile([128, BL], fp32, tag="gps", name="eps")
                for ncn in range(NC_CH):
                    s = slice(ncn * 512, (ncn + 1) * 512)
                    nc.tensor.matmul(
                        eps[:, s],
                        w1t_sb[:, mj * 128 : (mj + 1) * 128],
                        xT_sb[:, s],
                        start=True,
                        stop=True,
                    )
                et = workp.tile([128, BL], bf16, tag=f"g{mj}", name=f"embT{mj}")
                nc.scalar.activation(et[:], eps[:], AF.Relu)
                embT.append(et)

            # ---- setup: A[m] = 32*(We @ embT) + 32*bias_steady -> DRAM (bf16),
            #      with step 0 fused in (h0 = c0 = 0, xin = -0.8):
            #      gates0 = func(psum/32 + bias_d0) straight from psum. ----
            h_cur = [None] * K_TILES
            h8_cur = [None] * 2
            for p in range(2):
                h8_cur[p] = hpool.tile([128, 2, BL], fp8, tag=f"h8p{p}", name=f"h8p{p}_0")
            c_sb = [None] * K_TILES
            gact0 = {}
            STEP0_FUNC = {0: AF.Sigmoid, 2: AF.Tanh, 3: AF.Sigmoid}
            A_ORDER = [g * 4 + h for h in range(4) for g in (0, 2, 3)] + [4, 5, 6, 7]
            for m in A_ORDER:
                aps = psump.tile([128, BL], fp32, tag="gps", name="aps")
                for k in range(K_TILES):
                    for ncn in range(NC_CH):
                        s = slice(ncn * 512, (ncn + 1) * 512)
                        nc.tensor.matmul(
                            aps[:, s],
                            we_sb[k][:, m * 128 : (m + 1) * 128],
                            embT[k][:, s],
                            start=(k == 0),
                            stop=(k == K_TILES - 1),
                        )
                ast = apool.tile([128, BL], bf16, tag="astream", name="astage")
                nc.scalar.activation(
                    ast[:], aps[:], AF.Identity, bias=bias_st[:, m : m + 1]
                )
                nc.sync.dma_start(a_d[m], ast[:])
                gt, hid = divmod(m, 4)
                if gt in STEP0_FUNC:
                    g = workp.tile([128, BL], bf16, tag=f"g{gt}", name=f"g{gt}_0")
                    nc.scalar.activation(
                        g[:], aps[:], STEP0_FUNC[gt],
                        bias=bias_d0[:, m : m + 1], scale=ISC,
                    )
                    gact0[(gt, hid)] = g
                if gt == 3:
                    # o-gate staged: finish the hid's step-0 cell update
                    ct = statep.tile([128, BL], bf16, name=f"c{hid}")
                    nc.vector.tensor_tensor(
                        ct[:], gact0[(0, hid)][:], gact0[(2, hid)][:], OP.mult
                    )
                    tt = workp.tile([128, BL], bf16, tag="tt", bufs=1, name="t0")
                    nc.scalar.activation(tt[:], ct[:], AF.Tanh)
                    ht = hpool.tile([128, BL], bf16, tag=f"h{hid}", name=f"h{hid}_0")
                    nc.vector.tensor_tensor(ht[:], gact0[(3, hid)][:], tt[:], OP.mult)
                    nc.vector.tensor_copy(h8_cur[hid // 2][:, hid % 2, :], ht[:])
                    c_sb[hid] = ct
                    h_cur[hid] = ht

            # ---- steps 1..35 ----
            GATE_FUNC = {0: AF.Sigmoid, 1: AF.Sigmoid, 2: AF.Tanh, 3: AF.Sigmoid}

            def y_tile(t_out, h_in):
                """y = W3 @ h (+ b3) -> out[t_out] (bf16 h, unscaled weights)."""
                gps = psump.tile([128, BL], fp32, tag="gps", name="yps")
                for k in range(K_TILES):
                    for ncn in range(NC_CH):
                        s = slice(ncn * 512, (ncn + 1) * 512)
                        nc.tensor.matmul(
                            gps[:, s],
                            wbig_sb[k][:, Y_OFF : Y_OFF + 128],
                            h_in[k][:, s],
                            start=(k == 0),
                            stop=(k == K_TILES - 1),
                        )
                yr = workp.tile([1, BL], fp32, tag="yrow", bufs=2, name="yrow")
                nc.scalar.add(yr[:], gps[0:1, :], float(b3_val))
                nc.sync.dma_start(out_d[t_out : t_out + 1, :], yr[:])

            for t in range(1, n_steps):
                h_next = [None] * K_TILES
                h8_next = [None] * 2
                for p in range(2):
                    h8_next[p] = hpool.tile(
                        [128, 2, BL], fp8, tag=f"h8p{p}", name=f"h8p{p}_{t}"
                    )
                for hid in range(K_TILES):
                    use_identity = hid in ident_hids
                    split = split_last_cell and hid == K_TILES - 1
                    halves = (
                        [slice(0, BL // 2), slice(BL // 2, BL)]
                        if split
                        else [slice(0, BL)]
                    )
                    gact = {}
                    for gt in range(4):
                        m = gt * 4 + hid
                        gps = psump.tile([128, BL], fp32, tag="gps", name="gps")
                        ab = apool.tile([128, BL], bf16, tag="astream", name="abuf")
                        nc.sync.dma_start(ab[:], a_d[m])
                        if use_identity:
                            for ncn in range(NC_CH):
                                s = slice(ncn * 512, (ncn + 1) * 512)
                                nc.tensor.matmul(
                                    gps[:, s], ident_sb[:], ab[:, s],
                                    start=True, stop=False,
                                )
                        if gt in fp8_gates:
                            fi = fidx[m]
                            for p in range(2):
                                w = w8_sb[p][:, :, fi * 128 : (fi + 1) * 128]
                                for ncn in range(NC_CH):
                                    s = slice(ncn * 512, (ncn + 1) * 512)
                                    nc.tensor.matmul(
                                        gps[:, s],
                                        w,
                                        h8_cur[p][:, :, s],
                                        start=(p == 0 and not use_identity),
                                        stop=(p == 1),
                                        perf_mode=DR,
                                    )
                        else:
                            gi = gidx[m]
                            for k in range(K_TILES):
                                for ncn in range(NC_CH):
                                    s = slice(ncn * 512, (ncn + 1) * 512)
                                    nc.tensor.matmul(
                                        gps[:, s],
                                        wbig_sb[k][:, gi * 128 : (gi + 1) * 128],
                                        h_cur[k][:, s],
                                        start=(k == 0 and not use_identity),
                                        stop=(k == K_TILES - 1),
                                    )
                        g = workp.tile([128, BL], bf16, tag=f"g{gt}", name=f"g{gt}_{t}")
                        hs = halves if (split and gt == 3) else [slice(0, BL)]
                        if use_identity:
                            for s in hs:
                                nc.scalar.activation(
                                    g[:, s], gps[:, s], GATE_FUNC[gt], scale=ISC
                                )
                        else:
                            z = workp.tile(
                                [128, BL], bf16, tag=f"z{gt}", bufs=1, name=f"z{gt}"
                            )
                            for s in hs:
                                nc.vector.tensor_tensor(
                                    z[:, s], gps[:, s], ab[:, s], OP.add
                                )
                                nc.scalar.activation(
                                    g[:, s], z[:, s], GATE_FUNC[gt], scale=ISC
                                )
                        gact[gt] = g
                    # cell update for this hid tile (all-bf16 -> DVE 2x mode)
                    fc = workp.tile([128, BL], bf16, tag="fc", name="fc")
                    ig = workp.tile([128, BL], bf16, tag="ig", name="ig")
                    tt = workp.tile([128, BL], bf16, tag="tt", bufs=1, name="tt")
                    ht = hpool.tile([128, BL], bf16, tag=f"h{hid}", name=f"h{hid}_{t}")
                    eng_fc = nc.gpsimd if hid in gps_fc_hids else nc.vector
                    eng_ig = nc.gpsimd if hid in gps_ig_hids else nc.vector
                    p8, j8 = hid // 2, hid % 2
                    for s in halves:
                        eng_fc.tensor_tensor(fc[:, s], gact[1][:, s], c_sb[hid][:, s], OP.mult)
                        eng_ig.tensor_tensor(ig[:, s], gact[0][:, s], gact[2][:, s], OP.mult)
                        nc.vector.tensor_tensor(c_sb[hid][:, s], fc[:, s], ig[:, s], OP.add)
                        nc.scalar.activation(tt[:, s], c_sb[hid][:, s], AF.Tanh)
                        if hid == K_TILES - 1:
                            # hid3 sits on the cross-step critical chain: produce
                            # the fp8 h directly (one op earlier than via cast)
                            nc.vector.tensor_tensor(
                                h8_next[p8][:, j8, s], gact[3][:, s], tt[:, s], OP.mult
                            )
                            nc.vector.tensor_tensor(ht[:, s], gact[3][:, s], tt[:, s], OP.mult)
                        else:
                            nc.vector.tensor_tensor(ht[:, s], gact[3][:, s], tt[:, s], OP.mult)
                            nc.vector.tensor_copy(h8_next[p8][:, j8, s], ht[:, s])
                    h_next[hid] = ht
                # y_{t-1} from h_cur (the h this step's matmuls consumed);
                # emitted here it doubles as an h_t-independent PE buffer
                # that hides the hid3 cell-update tail at the step boundary.
                y_tile(t - 1, h_cur)
                h_cur = h_next
                h8_cur = h8_next

            # final output y_{n-1} from the last h
            y_tile(n_steps - 1, h_cur)

    nc.compile()
    return nc


def _prepare_inputs(x, W1, b1, Wih, bih, Whh, bhh, W3, b3,
                    fp8_gates: tuple = FP8_GATES):
    """Host-side exact weight folding (fp64) + per-core sharding."""
    wih_col = Wih[:, 511:512].astype(np.float64)  # [2048,1]
    Whh_eff = Whh.astype(np.float64) + wih_col @ W3.astype(np.float64)  # [2048,512]
    bias_steady = (
        bih.astype(np.float64) + bhh.astype(np.float64) + wih_col[:, 0] * float(b3[0])
    )
    # full step-0 bias (applied to the RAW A psum, pre-bias_steady)
    bias_d0 = bih.astype(np.float64) + bhh.astype(np.float64) - 0.8 * wih_col[:, 0]

    F_MS = [m for m in range(M_TILES) if m // 4 in fp8_gates]
    G_MS = [m for m in range(M_TILES) if m // 4 not in fp8_gates]
    NF, NGB = len(F_MS), len(G_MS)
    Y_OFF = NGB * 128

    WhhT = Whh_eff.T  # [512, 2048] lhsT layout

    # bf16 lhsT: g-gate columns (x32), then y column (x1)
    wbig = np.zeros((HID, Y_OFF + 128), np.float64)
    for i, m in enumerate(G_MS):
        wbig[:, i * 128 : (i + 1) * 128] = SC * WhhT[:, m * 128 : (m + 1) * 128]
    wbig[:, Y_OFF] = W3[0].astype(np.float64)

    # fp8 DoubleRow pairs: w8[p][r, j, i*128+c] = 32*WhhT[128*(2p+j)+r, F_MS[i]*128+c]
    w8 = []
    for p in range(2):
        wp = np.zeros((128, 2, NF * 128), np.float64)
        for j in range(2):
            k0 = 128 * (2 * p + j)
            for i, m in enumerate(F_MS):
                wp[:, j, i * 128 : (i + 1) * 128] = (
                    SC * WhhT[k0 : k0 + 128, m * 128 : (m + 1) * 128]
                )
        w8.append(wp.astype(np.float32).astype(F8E4))

    we = np.zeros((HID, NG), np.float32)
    we[:EMB, :] = SC * Wih[:, :EMB].T.astype(np.float64)  # row 511 zero

    w1t = np.zeros((24, 512), np.float32)
    w1t[:23, :EMB] = W1.T
    w1t[23, :EMB] = b1

    ident = np.eye(128, dtype=np.float32)

    bias_st_2d = (SC * bias_steady).reshape(M_TILES, 128).T.astype(np.float32)
    bias_d0_2d = bias_d0.reshape(M_TILES, 128).T.astype(np.float32)

    common = {
        "w1t": w1t.astype(BF16),
        "wbig": wbig.astype(np.float32).astype(BF16),
        "w8p0": w8[0],
        "w8p1": w8[1],
        "we": we.astype(BF16),
        "ident": ident.astype(BF16),
        "bias_st": np.ascontiguousarray(bias_st_2d),
        "bias_d0": np.ascontiguousarray(bias_d0_2d),
    }
    in_maps = []
    for c in range(N_CORES):
        xs = x[c * BL : (c + 1) * BL]  # [BL, 23]
        xT = np.ones((24, BL), np.float32)
        xT[:23, :] = xs.T
        m = dict(common)
        m["xT"] = np.ascontiguousarray(xT).astype(BF16)
        in_maps.append(m)
    return in_maps, float(b3[0])


def kernel(x, W1, b1, Wih, bih, Whh, bhh, W3, b3):
    from concourse.bass_utils import run_bass_kernel_spmd

    x = np.asarray(x, np.float32)
    in_maps, b3_val = _prepare_inputs(
        np.asarray(x, np.float32),
        np.asarray(W1, np.float32),
        np.asarray(b1, np.float32),
        np.asarray(Wih, np.float32),
        np.asarray(bih, np.float32),
        np.asarray(Whh, np.float32),
        np.asarray(bhh, np.float32),
        np.asarray(W3, np.float32),
        np.asarray(b3, np.float32),
    )
    nc = _build_program(b3_val)
    res = run_bass_kernel_spmd(nc, in_maps, list(range(N_CORES)))
    outs = [np.asarray(res.results[c]["out"]) for c in range(N_CORES)]  # [36, BL] each
    full = np.concatenate(outs, axis=1)  # [36, B]
    return full[:, :, None].astype(np.float32)  # [36, B, 1]


if __name__ == "__main__":
    rng = np.random.default_rng(0)
    ins = {
        "x": rng.standard_normal((B, 23), dtype=np.float32),
        "W1": rng.standard_normal((EMB, 23), dtype=np.float32) / np.sqrt(23),
        "b1": np.zeros(EMB, np.float32),
        "Wih": rng.standard_normal((NG, HID), dtype=np.float32) / np.sqrt(HID),
        "bih": np.zeros(NG, np.float32),
        "Whh": rng.standard_normal((NG, HID), dtype=np.float32) / np.sqrt(HID),
        "bhh": np.zeros(NG, np.float32),
        "W3": rng.standard_normal((1, HID), dtype=np.float32) / np.sqrt(HID),
        "b3": np.zeros(1, np.float32),
    }
    out = kernel(**ins)
    print("kernel output", out.shape, out.dtype, np.abs(out).max())


# revision 20
# speedup vs baseline: 1.0120x; 1.0120x over previous
"""Trainium2 Bass kernel for nn_ModelLSTM (36-step scalar-feedback LSTM).

Model (per reference):
    emb = relu(x @ W1.T + b1)                       # [B, 511], constant across steps
    x0 = -0.8; h0 = c0 = 0
    step t: inp = [emb, xin]                        # [B, 512]
            gates = inp @ Wih.T + bih + h @ Whh.T + bhh
            i,f,g,o = split(gates); c' = sig(f)*c + sig(i)*tanh(g); h' = sig(o)*tanh(c')
            y = h' @ W3.T + b3 ; xin' = y
    output ys = [36, B, 1]

Key restructurings:
  * xin_t = y_{t-1} = h_t @ W3.T + b3 folds into the recurrent weights:
    Whh_eff = Whh + Wih[:,511:] @ W3, bias folded similarly.
  * The emb contribution A = Wih[:,:511] @ emb.T is precomputed once and
    re-injected each step (identity-matmul into PSUM on PE, or DVE add).
  * fp8 (float8e4) DoubleRow matmuls for the i,f,o gate blocks (sigmoid's
    slope <= 1/4 attenuates the quantization error; the tanh g-block stays
    bf16). DoubleRow packs two 128-row contraction tiles per pass -> ~1.77x
    PE throughput on those tiles. Weights are pre-scaled by 32 to keep fp8
    values in the normal range; the gate activation applies scale=1/32.
    Host-side fp8 sweep predicts rel_err ~0.007 (vs 0.005 all-bf16).
  * c is stored bf16 so all cell-update DVE ops run in 2x packed mode.

Sharding: pure data-parallel over batch (16384 -> 8 cores x 2048). Weights
replicated. No collectives.

Layout: everything transposed - batch is the free dim, gate/hidden index is
the partition dim.
"""

import sys

for _p in ("/opt/trn_rl_repo",):
    if _p not in sys.path:
        sys.path.insert(0, _p)

import numpy as np
import ml_dtypes

BF16 = ml_dtypes.bfloat16
F8E4 = ml_dtypes.float8_e4m3  # TRN float8e4 (max 240)

N_CORES = 8
B = 16384
BL = B // N_CORES  # 2048 batch per core
HID = 512
EMB = 511
STEPS = 36
NG = 4 * HID  # 2048 gate rows
M_TILES = 16  # gate row tiles of 128
K_TILES = 4  # contraction tiles of 128 over HID
NC_CH = BL // 512  # 4 free-dim chunks of 512
SC = 32.0  # fp8 weight pre-scale (descaled in the gate activation)

FP8_GATES = (0, 1, 3)  # i, f, o via fp8 DoubleRow; g (tanh) stays bf16


def _build_program(
    b3_val: float,
    n_steps: int = STEPS,
    fp8_gates: tuple = FP8_GATES,
    ident_hids: tuple = (2, 3),
    gps_fc_hids: tuple = (0, 1),
    gps_ig_hids: tuple = (0,),
    split_last_cell: bool = False,
):
    """Build the Bass program.

    fp8_gates: gate types (0=i,1=f,2=g,3=o) whose matmul runs fp8 DoubleRow.
    ident_hids: hid groups whose A tiles are re-injected via identity matmul
      on PE; the rest add A on the Vector engine (PE vs DVE load balance).
    gps_fc_hids/gps_ig_hids: hids whose f*c / i*g products run on GpSimd.
    """
    import concourse.bass as bass
    import concourse.bacc as bacc
    import concourse.tile as tile
    from concourse import mybir

    fp32 = mybir.dt.float32
    bf16 = mybir.dt.bfloat16
    fp8 = mybir.dt.float8e4
    AF = mybir.ActivationFunctionType
    OP = mybir.AluOpType
    DR = mybir.MatmulPerfMode.DoubleRow
    ISC = 1.0 / SC

    F_MS = [m for m in range(M_TILES) if m // 4 in fp8_gates]  # fp8 m-tiles
    G_MS = [m for m in range(M_TILES) if m // 4 not in fp8_gates]  # bf16 m-tiles
    NF, NGB = len(F_MS), len(G_MS)
    fidx = {m: i for i, m in enumerate(F_MS)}
    gidx = {m: i for i, m in enumerate(G_MS)}
    Y_OFF = NGB * 128  # y column offset inside wbig

    nc = bacc.Bacc(
        "TRN2",
        target_bir_lowering=False,
        debug=False,
        num_devices=N_CORES,
    )

    # ---- DRAM I/O (per-core shapes) ----
    xT_d = nc.dram_tensor("xT", [24, BL], bf16, kind="ExternalInput")
    w1t_d = nc.dram_tensor("w1t", [24, 512], bf16, kind="ExternalInput")
    # bf16 lhsT: g-gate columns (x32) then the y column (x1), zero padded
    wbig_d = nc.dram_tensor("wbig", [HID, Y_OFF + 128], bf16, kind="ExternalInput")
    # fp8 DoubleRow weight pairs: pair p covers k-tiles (2p, 2p+1)
    w8_d = [
        nc.dram_tensor(f"w8p{p}", [128, 2, NF * 128], fp8, kind="ExternalInput")
        for p in range(2)
    ]
    we_d = nc.dram_tensor("we", [HID, NG], bf16, kind="ExternalInput")
    ident_d = nc.dram_tensor("ident", [128, 128], bf16, kind="ExternalInput")
    bias_st_d = nc.dram_tensor("bias_st", [128, M_TILES], fp32, kind="ExternalInput")
    bias_d0_d = nc.dram_tensor("bias_d0", [128, M_TILES], fp32, kind="ExternalInput")
    out_d = nc.dram_tensor("out", [STEPS, BL], fp32, kind="ExternalOutput")
    # scratch DRAM for the precomputed emb contribution A (streamed every step)
    a_d = nc.dram_tensor("a_scratch", [M_TILES, 128, BL], bf16, kind="Internal")

    with tile.TileContext(nc) as tc:
        with (
            tc.tile_pool(name="const", bufs=1) as constp,
            tc.tile_pool(name="state", bufs=1) as statep,
            tc.tile_pool(name="hpool", bufs=2) as hpool,
            tc.tile_pool(name="work", bufs=2) as workp,
            tc.tile_pool(name="astream", bufs=4) as apool,
            tc.tile_pool(name="psum", bufs=2, space=bass.MemorySpace.PSUM) as psump,
        ):
            # ---- load constants (emb inputs first so PE starts ASAP) ----
            xT_sb = workp.tile([24, BL], bf16, tag="fc", name="xT_sb")
            nc.sync.dma_start(xT_sb[:], xT_d[:])
            w1t_sb = workp.tile([24, 512], bf16, tag="ig", name="w1t_sb")
            nc.sync.dma_start(w1t_sb[:], w1t_d[:])
            wbig_sb = []
            for k in range(K_TILES):
                wt = constp.tile([128, Y_OFF + 128], bf16, name=f"wbig{k}")
                nc.sync.dma_start(wt[:], wbig_d[k * 128 : (k + 1) * 128, :])
                wbig_sb.append(wt)
            w8_sb = []
            for p in range(2):
                wt = constp.tile([128, 2, NF * 128], fp8, name=f"w8p{p}")
                nc.sync.dma_start(wt[:], w8_d[p][:])
                w8_sb.append(wt)
            ident_sb = constp.tile([128, 128], bf16, name="ident")
            nc.sync.dma_start(ident_sb[:], ident_d[:])
            bias_st = constp.tile([128, M_TILES], fp32, name="bias_st")
            nc.sync.dma_start(bias_st[:], bias_st_d[:])
            bias_d0 = constp.tile([128, M_TILES], fp32, name="bias_d0")
            nc.sync.dma_start(bias_d0[:], bias_d0_d[:])

            # ---- setup: emb = relu(x @ W1.T + b1) (transposed, bf16) ----
            we_sb = []
            for k in range(K_TILES):
                wet = hpool.tile([128, NG], bf16, tag=f"h{k}", name=f"we{k}")
                nc.sync.dma_start(wet[:], we_d[k * 128 : (k + 1) * 128, :])
                we_sb.append(wet)

            embT = []
            for mj in range(4):
                eps = psump.tile([128, BL], fp32, tag="gps", name="eps")
                for ncn in range(NC_CH):
                    s = slice(ncn * 512, (ncn + 1) * 512)
                    nc.tensor.matmul(
                        eps[:, s],
                        w1t_sb[:, mj * 128 : (mj + 1) * 128],
                        xT_sb[:, s],
                        start=True,
                        stop=True,
                    )
                et = workp.tile([128, BL], bf16, tag=f"g{mj}", name=f"embT{mj}")
                nc.scalar.activation(et[:], eps[:], AF.Relu)
                embT.append(et)

            # ---- setup: A[m] = 32*(We @ embT) + 32*bias_steady -> DRAM (bf16),
            #      with step 0 fused in (h0 = c0 = 0, xin = -0.8):
            #      gates0 = func(psum/32 + bias_d0) straight from psum. ----
            h_cur = [None] * K_TILES
            h8_cur = [None] * 2
            for p in range(2):
                h8_cur[p] = hpool.tile([128, 2, BL], fp8, tag=f"h8p{p}", name=f"h8p{p}_0")
            c_sb = [None] * K_TILES
            gact0 = {}
            STEP0_FUNC = {0: AF.Sigmoid, 2: AF.Tanh, 3: AF.Sigmoid}
            A_ORDER = [g * 4 + h for h in range(4) for g in (0, 2, 3)] + [4, 5, 6, 7]
            for m in A_ORDER:
                aps = psump.tile([128, BL], fp32, tag="gps", name="aps")
                for k in range(K_TILES):
                    for ncn in range(NC_CH):
                        s = slice(ncn * 512, (ncn + 1) * 512)
                        nc.tensor.matmul(
                            aps[:, s],
                            we_sb[k][:, m * 128 : (m + 1) * 128],
                            embT[k][:, s],
                            start=(k == 0),
                            stop=(k == K_TILES - 1),
                        )
                ast = apool.tile([128, BL], bf16, tag="astream", name="astage")
                nc.scalar.activation(
                    ast[:], aps[:], AF.Identity, bias=bias_st[:, m : m + 1]
                )
                nc.sync.dma_start(a_d[m], ast[:])
                gt, hid = divmod(m, 4)
                if gt in STEP0_FUNC:
                    g = workp.tile([128, BL], bf16, tag=f"g{gt}", name=f"g{gt}_0")
                    nc.scalar.activation(
                        g[:], aps[:], STEP0_FUNC[gt],
                        bias=bias_d0[:, m : m + 1], scale=ISC,
                    )
                    gact0[(gt, hid)] = g
                if gt == 3:
                    # o-gate staged: finish the hid's step-0 cell update
                    ct = statep.tile([128, BL], bf16, name=f"c{hid}")
                    nc.vector.tensor_tensor(
                        ct[:], gact0[(0, hid)][:], gact0[(2, hid)][:], OP.mult
                    )
                    tt = workp.tile([128, BL], bf16, tag="tt", bufs=1, name="t0")
                    nc.scalar.activation(tt[:], ct[:], AF.Tanh)
                    ht = hpool.tile([128, BL], bf16, tag=f"h{hid}", name=f"h{hid}_0")
                    nc.vector.tensor_tensor(ht[:], gact0[(3, hid)][:], tt[:], OP.mult)
                    nc.vector.tensor_copy(h8_cur[hid // 2][:, hid % 2, :], ht[:])
                    c_sb[hid] = ct
                    h_cur[hid] = ht

            # ---- steps 1..35 ----
            GATE_FUNC = {0: AF.Sigmoid, 1: AF.Sigmoid, 2: AF.Tanh, 3: AF.Sigmoid}

            def y_tile(t_out, h_in):
                """y = W3 @ h (+ b3) -> out[t_out] (bf16 h, unscaled weights)."""
                gps = psump.tile([128, BL], fp32, tag="gps", name="yps")
                for k in range(K_TILES):
                    for ncn in range(NC_CH):
                        s = slice(ncn * 512, (ncn + 1) * 512)
                        nc.tensor.matmul(
                            gps[:, s],
                            wbig_sb[k][:, Y_OFF : Y_OFF + 128],
                            h_in[k][:, s],
                            start=(k == 0),
                            stop=(k == K_TILES - 1),
                        )
                yr = workp.tile([1, BL], fp32, tag="yrow", bufs=2, name="yrow")
                nc.scalar.add(yr[:], gps[0:1, :], float(b3_val))
                nc.sync.dma_start(out_d[t_out : t_out + 1, :], yr[:])

            for t in range(1, n_steps):
                h_next = [None] * K_TILES
                h8_next = [None] * 2
                for p in range(2):
                    h8_next[p] = hpool.tile(
                        [128, 2, BL], fp8, tag=f"h8p{p}", name=f"h8p{p}_{t}"
                    )
                for hid in range(K_TILES):
                    use_identity = hid in ident_hids
                    split = split_last_cell and hid == K_TILES - 1
                    halves = (
                        [slice(0, BL // 2), slice(BL // 2, BL)]
                        if split
                        else [slice(0, BL)]
                    )
                    gact = {}
                    for gt in range(4):
                        m = gt * 4 + hid
                        gps = psump.tile([128, BL], fp32, tag="gps", name="gps")
                        ab = apool.tile([128, BL], bf16, tag="astream", name="abuf")
                        nc.sync.dma_start(ab[:], a_d[m])
                        if use_identity:
                            for ncn in range(NC_CH):
                                s = slice(ncn * 512, (ncn + 1) * 512)
                                nc.tensor.matmul(
                                    gps[:, s], ident_sb[:], ab[:, s],
                                    start=True, stop=False,
                                )
                        if gt in fp8_gates:
                            fi = fidx[m]
                            for p in range(2):
                                w = w8_sb[p][:, :, fi * 128 : (fi + 1) * 128]
                                for ncn in range(NC_CH):
                                    s = slice(ncn * 512, (ncn + 1) * 512)
                                    nc.tensor.matmul(
                                        gps[:, s],
                                        w,
                                        h8_cur[p][:, :, s],
                                        start=(p == 0 and not use_identity),
                                        stop=(p == 1),
                                        perf_mode=DR,
                                    )
                        else:
                            gi = gidx[m]
                            for k in range(K_TILES):
                                for ncn in range(NC_CH):
                                    s = slice(ncn * 512, (ncn + 1) * 512)
                                    nc.tensor.matmul(
                                        gps[:, s],
                                        wbig_sb[k][:, gi * 128 : (gi + 1) * 128],
                                        h_cur[k][:, s],
                                        start=(k == 0 and not use_identity),
                                        stop=(k == K_TILES - 1),
                                    )
                        g = workp.tile([128, BL], bf16, tag=f"g{gt}", name=f"g{gt}_{t}")
                        hs = halves if (split and gt == 3) else [slice(0, BL)]
                        if use_identity:
                            for s in hs:
                                nc.scalar.activation(
                                    g[:, s], gps[:, s], GATE_FUNC[gt], scale=ISC
                                )
                        else:
                            z = workp.tile(
                                [128, BL], bf16, tag=f"z{gt}", bufs=1, name=f"z{gt}"
                            )
                            for s in hs:
                                nc.vector.tensor_tensor(
                                    z[:, s], gps[:, s], ab[:, s], OP.add
                                )
                                nc.scalar.activation(
                                    g[:, s], z[:, s], GATE_FUNC[gt], scale=ISC
                                )
                        gact[gt] = g
                    # cell update for this hid tile (all-bf16 -> DVE 2x mode)
                    fc = workp.tile([128, BL], bf16, tag="fc", name="fc")
                    ig = workp.tile([128, BL], bf16, tag="ig", name="ig")
                    tt = workp.tile([128, BL], bf16, tag="tt", bufs=1, name="tt")
                    ht = hpool.tile([128, BL], bf16, tag=f"h{hid}", name=f"h{hid}_{t}")
                    eng_fc = nc.gpsimd if hid in gps_fc_hids else nc.vector
                    eng_ig = nc.gpsimd if hid in gps_ig_hids else nc.vector
                    p8, j8 = hid // 2, hid % 2
                    for s in halves:
                        eng_fc.tensor_tensor(fc[:, s], gact[1][:, s], c_sb[hid][:, s], OP.mult)
                        eng_ig.tensor_tensor(ig[:, s], gact[0][:, s], gact[2][:, s], OP.mult)
                        nc.vector.tensor_tensor(c_sb[hid][:, s], fc[:, s], ig[:, s], OP.add)
                        nc.scalar.activation(tt[:, s], c_sb[hid][:, s], AF.Tanh)
                        if hid == K_TILES - 1:
                            # hid3 sits on the cross-step critical chain: produce
                            # the fp8 h directly (one op earlier than via cast)
                            nc.vector.tensor_tensor(
                                h8_next[p8][:, j8, s], gact[3][:, s], tt[:, s], OP.mult
                            )
                            nc.vector.tensor_tensor(ht[:, s], gact[3][:, s], tt[:, s], OP.mult)
                        else:
                            nc.vector.tensor_tensor(ht[:, s], gact[3][:, s], tt[:, s], OP.mult)
                            nc.vector.tensor_copy(h8_next[p8][:, j8, s], ht[:, s])
                    h_next[hid] = ht
                # y_{t-1} from h_cur (the h this step's matmuls consumed);
                # emitted here it doubles as an h_t-independent PE buffer
                # that hides the hid3 cell-update tail at the step boundary.
                y_tile(t - 1, h_cur)
                h_cur = h_next
                h8_cur = h8_next

            # final output y_{n-1} from the last h
            y_tile(n_steps - 1, h_cur)

    nc.compile()
    return nc


def _prepare_inputs(x, W1, b1, Wih, bih, Whh, bhh, W3, b3,
                    fp8_gates: tuple = FP8_GATES):
    """Host-side exact weight folding (fp64) + per-core sharding."""
    wih_col = Wih[:, 511:512].astype(np.float64)  # [2048,1]
    Whh_eff = Whh.astype(np.float64) + wih_col @ W3.astype(np.float64)  # [2048,512]
    bias_steady = (
        bih.astype(np.float64) + bhh.astype(np.float64) + wih_col[:, 0] * float(b3[0])
    )
    # full step-0 bias (applied to the RAW A psum, pre-bias_steady)
    bias_d0 = bih.astype(np.float64) + bhh.astype(np.float64) - 0.8 * wih_col[:, 0]

    F_MS = [m for m in range(M_TILES) if m // 4 in fp8_gates]
    G_MS = [m for m in range(M_TILES) if m // 4 not in fp8_gates]
    NF, NGB = len(F_MS), len(G_MS)
    Y_OFF = NGB * 128

    WhhT = Whh_eff.T  # [512, 2048] lhsT layout

    # bf16 lhsT: g-gate columns (x32), then y column (x1)
    wbig = np.zeros((HID, Y_OFF + 128), np.float64)
    for i, m in enumerate(G_MS):
        wbig[:, i * 128 : (i + 1) * 128] = SC * WhhT[:, m * 128 : (m + 1) * 128]
    wbig[:, Y_OFF] = W3[0].astype(np.float64)

    # fp8 DoubleRow pairs: w8[p][r, j, i*128+c] = 32*WhhT[128*(2p+j)+r, F_MS[i]*128+c]
    w8 = []
    for p in range(2):
        wp = np.zeros((128, 2, NF * 128), np.float64)
        for j in range(2):
            k0 = 128 * (2 * p + j)
            for i, m in enumerate(F_MS):
                wp[:, j, i * 128 : (i + 1) * 128] = (
                    SC * WhhT[k0 : k0 + 128, m * 128 : (m + 1) * 128]
                )
        w8.append(wp.astype(np.float32).astype(F8E4))

    we = np.zeros((HID, NG), np.float32)
    we[:EMB, :] = SC * Wih[:, :EMB].T.astype(np.float64)  # row 511 zero

    w1t = np.zeros((24, 512), np.float32)
    w1t[:23, :EMB] = W1.T
    w1t[23, :EMB] = b1

    ident = np.eye(128, dtype=np.float32)

    bias_st_2d = (SC * bias_steady).reshape(M_TILES, 128).T.astype(np.float32)
    bias_d0_2d = bias_d0.reshape(M_TILES, 128).T.astype(np.float32)

    common = {
        "w1t": w1t.astype(BF16),
        "wbig": wbig.astype(np.float32).astype(BF16),
        "w8p0": w8[0],
        "w8p1": w8[1],
        "we": we.astype(BF16),
        "ident": ident.astype(BF16),
        "bias_st": np.ascontiguousarray(bias_st_2d),
        "bias_d0": np.ascontiguousarray(bias_d0_2d),
    }
    in_maps = []
    for c in range(N_CORES):
        xs = x[c * BL : (c + 1) * BL]  # [BL, 23]
        xT = np.ones((24, BL), np.float32)
        xT[:23, :] = xs.T
        m = dict(common)
        m["xT"] = np.ascontiguousarray(xT).astype(BF16)
        in_maps.append(m)
    return in_maps, float(b3[0])


def kernel(x, W1, b1, Wih, bih, Whh, bhh, W3, b3):
    from concourse.bass_utils import run_bass_kernel_spmd

    x = np.asarray(x, np.float32)
    in_maps, b3_val = _prepare_inputs(
        np.asarray(x, np.float32),
        np.asarray(W1, np.float32),
        np.asarray(b1, np.float32),
        np.asarray(Wih, np.float32),
        np.asarray(bih, np.float32),
        np.asarray(Whh, np.float32),
        np.asarray(bhh, np.float32),
        np.asarray(W3, np.float32),
        np.asarray(b3, np.float32),
    )
    nc = _build_program(b3_val)
    res = run_bass_kernel_spmd(nc, in_maps, list(range(N_CORES)))
    outs = [np.asarray(res.results[c]["out"]) for c in range(N_CORES)]  # [36, BL] each
    full = np.concatenate(outs, axis=1)  # [36, B]
    return full[:, :, None].astype(np.float32)  # [36, B, 1]


if __name__ == "__main__":
    rng = np.random.default_rng(0)
    ins = {
        "x": rng.standard_normal((B, 23), dtype=np.float32),
        "W1": rng.standard_normal((EMB, 23), dtype=np.float32) / np.sqrt(23),
        "b1": np.zeros(EMB, np.float32),
        "Wih": rng.standard_normal((NG, HID), dtype=np.float32) / np.sqrt(HID),
        "bih": np.zeros(NG, np.float32),
        "Whh": rng.standard_normal((NG, HID), dtype=np.float32) / np.sqrt(HID),
        "bhh": np.zeros(NG, np.float32),
        "W3": rng.standard_normal((1, HID), dtype=np.float32) / np.sqrt(HID),
        "b3": np.zeros(1, np.float32),
    }
    out = kernel(**ins)
    print("kernel output", out.shape, out.dtype, np.abs(out).max())


# revision 21
# speedup vs baseline: 1.0403x; 1.0279x over previous
"""Trainium2 Bass kernel for nn_ModelLSTM (36-step scalar-feedback LSTM).

Model (per reference):
    emb = relu(x @ W1.T + b1)                       # [B, 511], constant across steps
    x0 = -0.8; h0 = c0 = 0
    step t: inp = [emb, xin]                        # [B, 512]
            gates = inp @ Wih.T + bih + h @ Whh.T + bhh
            i,f,g,o = split(gates); c' = sig(f)*c + sig(i)*tanh(g); h' = sig(o)*tanh(c')
            y = h' @ W3.T + b3 ; xin' = y
    output ys = [36, B, 1]

Key restructurings:
  * xin_t = y_{t-1} = h_t @ W3.T + b3 folds into the recurrent weights:
    Whh_eff = Whh + Wih[:,511:] @ W3, bias folded similarly.
  * The emb contribution A = Wih[:,:511] @ emb.T is precomputed once and
    re-injected each step (identity-matmul into PSUM on PE, or DVE add).
  * fp8 (float8e4) DoubleRow matmuls for the i,f,o gate blocks (sigmoid's
    slope <= 1/4 attenuates the quantization error; the tanh g-block stays
    bf16). DoubleRow packs two 128-row contraction tiles per pass -> ~1.77x
    PE throughput on those tiles. Weights are pre-scaled by 32 to keep fp8
    values in the normal range; the gate activation applies scale=1/32.
    Host-side fp8 sweep predicts rel_err ~0.007 (vs 0.005 all-bf16).
  * c is stored bf16 so all cell-update DVE ops run in 2x packed mode.

Sharding: pure data-parallel over batch (16384 -> 8 cores x 2048). Weights
replicated. No collectives.

Layout: everything transposed - batch is the free dim, gate/hidden index is
the partition dim.
"""

import sys

for _p in ("/opt/trn_rl_repo",):
    if _p not in sys.path:
        sys.path.insert(0, _p)

import numpy as np
import ml_dtypes

BF16 = ml_dtypes.bfloat16
F8E4 = ml_dtypes.float8_e4m3  # TRN float8e4 (max 240)

N_CORES = 8
B = 16384
BL = B // N_CORES  # 2048 batch per core
HID = 512
EMB = 511
STEPS = 36
NG = 4 * HID  # 2048 gate rows
M_TILES = 16  # gate row tiles of 128
K_TILES = 4  # contraction tiles of 128 over HID
NC_CH = BL // 512  # 4 free-dim chunks of 512
SC = 32.0  # fp8 weight pre-scale (descaled in the gate activation)

FP8_GATES = (0, 1, 3)  # i, f, o via fp8 DoubleRow; g (tanh) stays bf16


def _build_program(
    b3_val: float,
    n_steps: int = STEPS,
    fp8_gates: tuple = FP8_GATES,
    ident_hids: tuple = (2, 3),
    gps_fc_hids: tuple = (0, 1),
    gps_ig_hids: tuple = (0,),
    split_last_cell: bool = True,
):
    """Build the Bass program.

    fp8_gates: gate types (0=i,1=f,2=g,3=o) whose matmul runs fp8 DoubleRow.
    ident_hids: hid groups whose A tiles are re-injected via identity matmul
      on PE; the rest add A on the Vector engine (PE vs DVE load balance).
    gps_fc_hids/gps_ig_hids: hids whose f*c / i*g products run on GpSimd.
    """
    import concourse.bass as bass
    import concourse.bacc as bacc
    import concourse.tile as tile
    from concourse import mybir

    fp32 = mybir.dt.float32
    bf16 = mybir.dt.bfloat16
    fp8 = mybir.dt.float8e4
    AF = mybir.ActivationFunctionType
    OP = mybir.AluOpType
    DR = mybir.MatmulPerfMode.DoubleRow
    ISC = 1.0 / SC

    F_MS = [m for m in range(M_TILES) if m // 4 in fp8_gates]  # fp8 m-tiles
    G_MS = [m for m in range(M_TILES) if m // 4 not in fp8_gates]  # bf16 m-tiles
    NF, NGB = len(F_MS), len(G_MS)
    fidx = {m: i for i, m in enumerate(F_MS)}
    gidx = {m: i for i, m in enumerate(G_MS)}
    Y_OFF = NGB * 128  # y column offset inside wbig

    nc = bacc.Bacc(
        "TRN2",
        target_bir_lowering=False,
        debug=False,
        num_devices=N_CORES,
    )

    # ---- DRAM I/O (per-core shapes) ----
    xT_d = nc.dram_tensor("xT", [24, BL], bf16, kind="ExternalInput")
    w1t_d = nc.dram_tensor("w1t", [24, 512], bf16, kind="ExternalInput")
    # bf16 lhsT: g-gate columns (x32) then the y column (x1), zero padded
    wbig_d = nc.dram_tensor("wbig", [HID, Y_OFF + 128], bf16, kind="ExternalInput")
    # fp8 DoubleRow weight pairs: pair p covers k-tiles (2p, 2p+1)
    w8_d = [
        nc.dram_tensor(f"w8p{p}", [128, 2, NF * 128], fp8, kind="ExternalInput")
        for p in range(2)
    ]
    we_d = nc.dram_tensor("we", [HID, NG], bf16, kind="ExternalInput")
    ident_d = nc.dram_tensor("ident", [128, 128], bf16, kind="ExternalInput")
    bias_st_d = nc.dram_tensor("bias_st", [128, M_TILES], fp32, kind="ExternalInput")
    bias_d0_d = nc.dram_tensor("bias_d0", [128, M_TILES], fp32, kind="ExternalInput")
    out_d = nc.dram_tensor("out", [STEPS, BL], fp32, kind="ExternalOutput")
    # scratch DRAM for the precomputed emb contribution A (streamed every step)
    a_d = nc.dram_tensor("a_scratch", [M_TILES, 128, BL], bf16, kind="Internal")

    with tile.TileContext(nc) as tc:
        with (
            tc.tile_pool(name="const", bufs=1) as constp,
            tc.tile_pool(name="state", bufs=1) as statep,
            tc.tile_pool(name="hpool", bufs=2) as hpool,
            tc.tile_pool(name="work", bufs=2) as workp,
            tc.tile_pool(name="astream", bufs=4) as apool,
            tc.tile_pool(name="psum", bufs=2, space=bass.MemorySpace.PSUM) as psump,
        ):
            # ---- load constants (emb inputs first so PE starts ASAP) ----
            xT_sb = workp.tile([24, BL], bf16, tag="fc", name="xT_sb")
            nc.sync.dma_start(xT_sb[:], xT_d[:])
            w1t_sb = workp.tile([24, 512], bf16, tag="ig", name="w1t_sb")
            nc.sync.dma_start(w1t_sb[:], w1t_d[:])
            wbig_sb = []
            for k in range(K_TILES):
                wt = constp.tile([128, Y_OFF + 128], bf16, name=f"wbig{k}")
                nc.sync.dma_start(wt[:], wbig_d[k * 128 : (k + 1) * 128, :])
                wbig_sb.append(wt)
            w8_sb = []
            for p in range(2):
                wt = constp.tile([128, 2, NF * 128], fp8, name=f"w8p{p}")
                nc.sync.dma_start(wt[:], w8_d[p][:])
                w8_sb.append(wt)
            ident_sb = constp.tile([128, 128], bf16, name="ident")
            nc.sync.dma_start(ident_sb[:], ident_d[:])
            bias_st = constp.tile([128, M_TILES], fp32, name="bias_st")
            nc.sync.dma_start(bias_st[:], bias_st_d[:])
            bias_d0 = constp.tile([128, M_TILES], fp32, name="bias_d0")
            nc.sync.dma_start(bias_d0[:], bias_d0_d[:])

            # ---- setup: emb = relu(x @ W1.T + b1) (transposed, bf16) ----
            we_sb = []
            for k in range(K_TILES):
                wet = hpool.tile([128, NG], bf16, tag=f"h{k}", name=f"we{k}")
                nc.sync.dma_start(wet[:], we_d[k * 128 : (k + 1) * 128, :])
                we_sb.append(wet)

            embT = []
            for mj in range(4):
                eps = psump.tile([128, BL], fp32, tag="gps", name="eps")
                for ncn in range(NC_CH):
                    s = slice(ncn * 512, (ncn + 1) * 512)
                    nc.tensor.matmul(
                        eps[:, s],
                        w1t_sb[:, mj * 128 : (mj + 1) * 128],
                        xT_sb[:, s],
                        start=True,
                        stop=True,
                    )
                et = workp.tile([128, BL], bf16, tag=f"g{mj}", name=f"embT{mj}")
                nc.scalar.activation(et[:], eps[:], AF.Relu)
                embT.append(et)

            # ---- setup: A[m] = 32*(We @ embT) + 32*bias_steady -> DRAM (bf16),
            #      with step 0 fused in (h0 = c0 = 0, xin = -0.8):
            #      gates0 = func(psum/32 + bias_d0) straight from psum. ----
            h_cur = [None] * K_TILES
            h8_cur = [None] * 2
            for p in range(2):
                h8_cur[p] = hpool.tile([128, 2, BL], fp8, tag=f"h8p{p}", name=f"h8p{p}_0")
            c_sb = [None] * K_TILES
            gact0 = {}
            STEP0_FUNC = {0: AF.Sigmoid, 2: AF.Tanh, 3: AF.Sigmoid}
            A_ORDER = [g * 4 + h for h in range(4) for g in (0, 2, 3)] + [4, 5, 6, 7]
            for m in A_ORDER:
                aps = psump.tile([128, BL], fp32, tag="gps", name="aps")
                for k in range(K_TILES):
                    for ncn in range(NC_CH):
                        s = slice(ncn * 512, (ncn + 1) * 512)
                        nc.tensor.matmul(
                            aps[:, s],
                            we_sb[k][:, m * 128 : (m + 1) * 128],
                            embT[k][:, s],
                            start=(k == 0),
                            stop=(k == K_TILES - 1),
                        )
                ast = apool.tile([128, BL], bf16, tag="astream", name="astage")
                nc.scalar.activation(
                    ast[:], aps[:], AF.Identity, bias=bias_st[:, m : m + 1]
                )
                nc.sync.dma_start(a_d[m], ast[:])
                gt, hid = divmod(m, 4)
                if gt in STEP0_FUNC:
                    g = workp.tile([128, BL], bf16, tag=f"g{gt}", name=f"g{gt}_0")
                    nc.scalar.activation(
                        g[:], aps[:], STEP0_FUNC[gt],
                        bias=bias_d0[:, m : m + 1], scale=ISC,
                    )
                    gact0[(gt, hid)] = g
                if gt == 3:
                    # o-gate staged: finish the hid's step-0 cell update
                    ct = statep.tile([128, BL], bf16, name=f"c{hid}")
                    nc.vector.tensor_tensor(
                        ct[:], gact0[(0, hid)][:], gact0[(2, hid)][:], OP.mult
                    )
                    tt = workp.tile([128, BL], bf16, tag="tt", bufs=1, name="t0")
                    nc.scalar.activation(tt[:], ct[:], AF.Tanh)
                    ht = hpool.tile([128, BL], bf16, tag=f"h{hid}", name=f"h{hid}_0")
                    nc.vector.tensor_tensor(ht[:], gact0[(3, hid)][:], tt[:], OP.mult)
                    nc.vector.tensor_copy(h8_cur[hid // 2][:, hid % 2, :], ht[:])
                    c_sb[hid] = ct
                    h_cur[hid] = ht

            # ---- steps 1..35 ----
            GATE_FUNC = {0: AF.Sigmoid, 1: AF.Sigmoid, 2: AF.Tanh, 3: AF.Sigmoid}

            def y_tile(t_out, h_in):
                """y = W3 @ h (+ b3) -> out[t_out] (bf16 h, unscaled weights)."""
                gps = psump.tile([128, BL], fp32, tag="gps", name="yps")
                for k in range(K_TILES):
                    for ncn in range(NC_CH):
                        s = slice(ncn * 512, (ncn + 1) * 512)
                        nc.tensor.matmul(
                            gps[:, s],
                            wbig_sb[k][:, Y_OFF : Y_OFF + 128],
                            h_in[k][:, s],
                            start=(k == 0),
                            stop=(k == K_TILES - 1),
                        )
                yr = workp.tile([1, BL], fp32, tag="yrow", bufs=2, name="yrow")
                nc.scalar.add(yr[:], gps[0:1, :], float(b3_val))
                nc.sync.dma_start(out_d[t_out : t_out + 1, :], yr[:])

            for t in range(1, n_steps):
                h_next = [None] * K_TILES
                h8_next = [None] * 2
                for p in range(2):
                    h8_next[p] = hpool.tile(
                        [128, 2, BL], fp8, tag=f"h8p{p}", name=f"h8p{p}_{t}"
                    )
                for hid in range(K_TILES):
                    use_identity = hid in ident_hids
                    split = split_last_cell and hid == K_TILES - 1
                    halves = (
                        [slice(0, BL // 2), slice(BL // 2, BL)]
                        if split
                        else [slice(0, BL)]
                    )
                    gact = {}
                    for gt in (1, 0, 2, 3):
                        m = gt * 4 + hid
                        gps = psump.tile([128, BL], fp32, tag="gps", name="gps")
                        ab = apool.tile([128, BL], bf16, tag="astream", name="abuf")
                        nc.sync.dma_start(ab[:], a_d[m])
                        if use_identity:
                            for ncn in range(NC_CH):
                                s = slice(ncn * 512, (ncn + 1) * 512)
                                nc.tensor.matmul(
                                    gps[:, s], ident_sb[:], ab[:, s],
                                    start=True, stop=False,
                                )
                        if gt in fp8_gates:
                            fi = fidx[m]
                            for p in range(2):
                                w = w8_sb[p][:, :, fi * 128 : (fi + 1) * 128]
                                for ncn in range(NC_CH):
                                    s = slice(ncn * 512, (ncn + 1) * 512)
                                    nc.tensor.matmul(
                                        gps[:, s],
                                        w,
                                        h8_cur[p][:, :, s],
                                        start=(p == 0 and not use_identity),
                                        stop=(p == 1),
                                        perf_mode=DR,
                                    )
                        else:
                            gi = gidx[m]
                            for k in range(K_TILES):
                                for ncn in range(NC_CH):
                                    s = slice(ncn * 512, (ncn + 1) * 512)
                                    nc.tensor.matmul(
                                        gps[:, s],
                                        wbig_sb[k][:, gi * 128 : (gi + 1) * 128],
                                        h_cur[k][:, s],
                                        start=(k == 0 and not use_identity),
                                        stop=(k == K_TILES - 1),
                                    )
                        g = workp.tile([128, BL], bf16, tag=f"g{gt}", name=f"g{gt}_{t}")
                        hs = halves if (split and gt == 3) else [slice(0, BL)]
                        if use_identity:
                            for s in hs:
                                nc.scalar.activation(
                                    g[:, s], gps[:, s], GATE_FUNC[gt], scale=ISC
                                )
                        else:
                            z = workp.tile(
                                [128, BL], bf16, tag=f"z{gt}", bufs=1, name=f"z{gt}"
                            )
                            for s in hs:
                                nc.vector.tensor_tensor(
                                    z[:, s], gps[:, s], ab[:, s], OP.add
                                )
                                nc.scalar.activation(
                                    g[:, s], z[:, s], GATE_FUNC[gt], scale=ISC
                                )
                        gact[gt] = g
                    # cell update for this hid tile (all-bf16 -> DVE 2x mode)
                    fc = workp.tile([128, BL], bf16, tag="fc", name="fc")
                    ig = workp.tile([128, BL], bf16, tag="ig", name="ig")
                    tt = workp.tile([128, BL], bf16, tag="tt", bufs=1, name="tt")
                    ht = hpool.tile([128, BL], bf16, tag=f"h{hid}", name=f"h{hid}_{t}")
                    eng_fc = nc.gpsimd if hid in gps_fc_hids else nc.vector
                    eng_ig = nc.gpsimd if hid in gps_ig_hids else nc.vector
                    p8, j8 = hid // 2, hid % 2
                    for s in halves:
                        eng_fc.tensor_tensor(fc[:, s], gact[1][:, s], c_sb[hid][:, s], OP.mult)
                        eng_ig.tensor_tensor(ig[:, s], gact[0][:, s], gact[2][:, s], OP.mult)
                        nc.vector.tensor_tensor(c_sb[hid][:, s], fc[:, s], ig[:, s], OP.add)
                        nc.scalar.activation(tt[:, s], c_sb[hid][:, s], AF.Tanh)
                        nc.vector.tensor_tensor(ht[:, s], gact[3][:, s], tt[:, s], OP.mult)
                        nc.vector.tensor_copy(h8_next[p8][:, j8, s], ht[:, s])
                    h_next[hid] = ht
                # y_{t-1} from h_cur (the h this step's matmuls consumed);
                # emitted here it doubles as an h_t-independent PE buffer
                # that hides the hid3 cell-update tail at the step boundary.
                y_tile(t - 1, h_cur)
                h_cur = h_next
                h8_cur = h8_next

            # final output y_{n-1} from the last h
            y_tile(n_steps - 1, h_cur)

    nc.compile()
    return nc


def _prepare_inputs(x, W1, b1, Wih, bih, Whh, bhh, W3, b3,
                    fp8_gates: tuple = FP8_GATES):
    """Host-side exact weight folding (fp64) + per-core sharding."""
    wih_col = Wih[:, 511:512].astype(np.float64)  # [2048,1]
    Whh_eff = Whh.astype(np.float64) + wih_col @ W3.astype(np.float64)  # [2048,512]
    bias_steady = (
        bih.astype(np.float64) + bhh.astype(np.float64) + wih_col[:, 0] * float(b3[0])
    )
    # full step-0 bias (applied to the RAW A psum, pre-bias_steady)
    bias_d0 = bih.astype(np.float64) + bhh.astype(np.float64) - 0.8 * wih_col[:, 0]

    F_MS = [m for m in range(M_TILES) if m // 4 in fp8_gates]
    G_MS = [m for m in range(M_TILES) if m // 4 not in fp8_gates]
    NF, NGB = len(F_MS), len(G_MS)
    Y_OFF = NGB * 128

    WhhT = Whh_eff.T  # [512, 2048] lhsT layout

    # bf16 lhsT: g-gate columns (x32), then y column (x1)
    wbig = np.zeros((HID, Y_OFF + 128), np.float64)
    for i, m in enumerate(G_MS):
        wbig[:, i * 128 : (i + 1) * 128] = SC * WhhT[:, m * 128 : (m + 1) * 128]
    wbig[:, Y_OFF] = W3[0].astype(np.float64)

    # fp8 DoubleRow pairs: w8[p][r, j, i*128+c] = 32*WhhT[128*(2p+j)+r, F_MS[i]*128+c]
    w8 = []
    for p in range(2):
        wp = np.zeros((128, 2, NF * 128), np.float64)
        for j in range(2):
            k0 = 128 * (2 * p + j)
            for i, m in enumerate(F_MS):
                wp[:, j, i * 128 : (i + 1) * 128] = (
                    SC * WhhT[k0 : k0 + 128, m * 128 : (m + 1) * 128]
                )
        w8.append(wp.astype(np.float32).astype(F8E4))

    we = np.zeros((HID, NG), np.float32)
    we[:EMB, :] = SC * Wih[:, :EMB].T.astype(np.float64)  # row 511 zero

    w1t = np.zeros((24, 512), np.float32)
    w1t[:23, :EMB] = W1.T
    w1t[23, :EMB] = b1

    ident = np.eye(128, dtype=np.float32)

    bias_st_2d = (SC * bias_steady).reshape(M_TILES, 128).T.astype(np.float32)
    bias_d0_2d = bias_d0.reshape(M_TILES, 128).T.astype(np.float32)

    common = {
        "w1t": w1t.astype(BF16),
        "wbig": wbig.astype(np.float32).astype(BF16),
        "w8p0": w8[0],
        "w8p1": w8[1],
        "we": we.astype(BF16),
        "ident": ident.astype(BF16),
        "bias_st": np.ascontiguousarray(bias_st_2d),
        "bias_d0": np.ascontiguousarray(bias_d0_2d),
    }
    in_maps = []
    for c in range(N_CORES):
        xs = x[c * BL : (c + 1) * BL]  # [BL, 23]
        xT = np.ones((24, BL), np.float32)
        xT[:23, :] = xs.T
        m = dict(common)
        m["xT"] = np.ascontiguousarray(xT).astype(BF16)
        in_maps.append(m)
    return in_maps, float(b3[0])


def kernel(x, W1, b1, Wih, bih, Whh, bhh, W3, b3):
    from concourse.bass_utils import run_bass_kernel_spmd

    x = np.asarray(x, np.float32)
    in_maps, b3_val = _prepare_inputs(
        np.asarray(x, np.float32),
        np.asarray(W1, np.float32),
        np.asarray(b1, np.float32),
        np.asarray(Wih, np.float32),
        np.asarray(bih, np.float32),
        np.asarray(Whh, np.float32),
        np.asarray(bhh, np.float32),
        np.asarray(W3, np.float32),
        np.asarray(b3, np.float32),
    )
    nc = _build_program(b3_val)
    res = run_bass_kernel_spmd(nc, in_maps, list(range(N_CORES)))
    outs = [np.asarray(res.results[c]["out"]) for c in range(N_CORES)]  # [36, BL] each
    full = np.concatenate(outs, axis=1)  # [36, B]
    return full[:, :, None].astype(np.float32)  # [36, B, 1]


if __name__ == "__main__":
    rng = np.random.default_rng(0)
    ins = {
        "x": rng.standard_normal((B, 23), dtype=np.float32),
        "W1": rng.standard_normal((EMB, 23), dtype=np.float32) / np.sqrt(23),
        "b1": np.zeros(EMB, np.float32),
        "Wih": rng.standard_normal((NG, HID), dtype=np.float32) / np.sqrt(HID),
        "bih": np.zeros(NG, np.float32),
        "Whh": rng.standard_normal((NG, HID), dtype=np.float32) / np.sqrt(HID),
        "bhh": np.zeros(NG, np.float32),
        "W3": rng.standard_normal((1, HID), dtype=np.float32) / np.sqrt(HID),
        "b3": np.zeros(1, np.float32),
    }
    out = kernel(**ins)
    print("kernel output", out.shape, out.dtype, np.abs(out).max())
